# revision 15
# baseline (speedup 1.0000x reference)
"""AdaptiveLocalPositionEmbedding Trainium2 kernel (8 NeuronCores, data parallel).

out[b,s,:] = x[b,s,:] + pos_emb[b,s,:] where pos_emb is:
  - control_emb[s]            if s < 4 and no start-token segment covers s
  - sequence_emb[s - last]    if a start token (>= pos 4) precedes s and rel < 1003
  - 0                         otherwise
`last` = latest position <= s with input_ids == start_token_id (at pos >= 4).

Device work per core (2 batch rows): segment scan (cummax) over the start-token
mask, per-token index computation, indirect-DMA gather of 2KB table rows, add.
Host work: dtype casts, equality mask vs. the runtime scalar start_token_id,
table concat, shard/unshard.
"""

import os
import sys

import numpy as np

for _p in ("/opt/trn_rl_repo",):
    if _p not in sys.path:
        sys.path.insert(0, _p)

from concourse import bacc, bass, mybir
from concourse.bass_utils import run_bass_kernel_spmd
from concourse.tile import TileContext

B, S, D = 16, 2048, 512
N_CORES = 8
B_SH = B // N_CORES            # 2 batch rows per core
TOK = B_SH * S                 # 4096 tokens per core
NT = TOK // 128                # 32 tiles of 128 tokens
NQ = 16                        # 128-token blocks per batch row (S / 128)
N_CTRL = 4
N_SEQ = 1003
ZERO_ROW = N_CTRL + N_SEQ      # 1007
TBL = ZERO_ROW + 1             # 1008 rows
F32 = mybir.dt.float32
I32 = mybir.dt.int32

_CACHE = {}


def _ensure_ntff_hook():
    """The agent image's antenv package lacks axon_hooks, so NTFF tracing
    silently degrades. Synthesize the module and register the boot script's
    ctypes-based profile hook so trace=True yields exec_time_ns."""
    if "antenv.axon_hooks" in sys.modules:
        return
    try:
        import types

        import antenv
        from trn_agent_boot.trn_boot import _ntff_profile_via_ctypes

        mod = types.ModuleType("antenv.axon_hooks")
        mod._hook = None

        def set_axon_ntff_profile_hook(h):
            mod._hook = h

        def get_axon_ntff_profile_hook():
            return mod._hook

        mod.set_axon_ntff_profile_hook = set_axon_ntff_profile_hook
        mod.get_axon_ntff_profile_hook = get_axon_ntff_profile_hook
        sys.modules["antenv.axon_hooks"] = mod
        antenv.axon_hooks = mod
        mod._hook = _ntff_profile_via_ctypes("/opt/axon/libaxon_pjrt.so")
    except Exception as e:  # tracing degrades; run still works
        print(f"NTFF hook registration failed: {e}", file=sys.stderr)


def _build_bass():
    nc = bacc.Bacc()
    x_h = nc.dram_tensor("x", [TOK, D], F32, kind="ExternalInput")
    # packed small inputs: [:, 0:128]=start mask, [:,128:256]=s+1,
    # [:,256:384]=base idx, [:,384:416]=32x32 identity, [:,416:417]=1.0
    cst_h = nc.dram_tensor("consts", [2 * NQ, 417], F32, kind="ExternalInput")
    table_h = nc.dram_tensor("table", [TBL, D], F32, kind="ExternalInput")
    out_h = nc.dram_tensor("out", [TOK, D], F32, kind="ExternalOutput")

    QP = 2 * NQ  # 32 partitions used by the index pipeline

    with TileContext(nc) as tc:
        with (
            tc.tile_pool(name="const", bufs=1) as cpool,
            tc.tile_pool(name="work", bufs=8) as wpool,
            tc.tile_pool(name="psum", bufs=1, space="PSUM") as ppool,
        ):
            # ---- constants / small inputs (single DMA → single wait) ----
            cst = cpool.tile([QP, 417], F32)
            nc.gpsimd.dma_start(out=cst[:], in_=cst_h[:])
            sm = cst[:, 0:128]
            svalp1 = cst[:, 128:256]
            basei = cst[:, 256:384]
            # identities staged via DVE so matmuls wait on one semaphore only
            id32s = cpool.tile([QP, QP], F32)
            id1s = cpool.tile([1, 1], F32)
            nc.vector.tensor_copy(out=id32s[:], in_=cst[:, 384:416])
            nc.vector.tensor_copy(out=id1s[:], in_=cst[0:1, 416:417])
            id32t = id32s[:]
            id1t = id1s[:]

            # ---- marker = sm * (s+1) - 1  (s where start token, else -1) ----
            sA = cpool.tile([QP, 128], F32)
            sB = cpool.tile([QP, 128], F32)
            nc.vector.tensor_tensor(out=sA[:], in0=sm, in1=svalp1,
                                    op=mybir.AluOpType.mult)
            nc.vector.tensor_scalar_add(out=sA[:], in0=sA[:], scalar1=-1.0)

            # ---- inclusive cummax along free dim (within each 128 block) ----
            cur, nxt = sA, sB
            for k in (1, 2, 4, 8, 16, 32, 64):
                nc.vector.tensor_copy(out=nxt[:, :k], in_=cur[:, :k])
                nc.vector.tensor_tensor(out=nxt[:, k:], in0=cur[:, k:],
                                        in1=cur[:, : 128 - k],
                                        op=mybir.AluOpType.max)
                cur, nxt = nxt, cur
            # cur[q, p] = max over p' <= p of marker(q, p')

            # ---- cross-block exclusive cummax (per batch row) ----
            mbT_ps = ppool.tile([1, QP], F32, space="PSUM")
            nc.tensor.matmul(out=mbT_ps[:], lhsT=cur[:, 127:128], rhs=id32t,
                             start=True, stop=True)
            ex = cpool.tile([1, QP], F32)
            ex2 = cpool.tile([1, QP], F32)
            nc.vector.memset(ex[:], -1.0)
            # exclusive shift within each 16-block half
            nc.vector.tensor_copy(out=ex[:, 1:NQ], in_=mbT_ps[:, 0:NQ - 1])
            nc.vector.tensor_copy(out=ex[:, NQ + 1:QP], in_=mbT_ps[:, NQ:QP - 1])
            curX, nxtX = ex, ex2
            for k in (1, 2, 4, 8):
                for h in (0, NQ):
                    nc.vector.tensor_copy(out=nxtX[:, h:h + k],
                                          in_=curX[:, h:h + k])
                    nc.vector.tensor_tensor(out=nxtX[:, h + k:h + NQ],
                                            in0=curX[:, h + k:h + NQ],
                                            in1=curX[:, h:h + NQ - k],
                                            op=mybir.AluOpType.max)
                curX, nxtX = nxtX, curX
            pref_ps = ppool.tile([QP, 1], F32, space="PSUM")
            nc.tensor.matmul(out=pref_ps[:], lhsT=curX[:], rhs=id1t,
                             start=True, stop=True)
            pref = cpool.tile([QP, 1], F32)
            nc.vector.tensor_copy(out=pref[:], in_=pref_ps[:])

            # ---- last_start, rel, validity, final table index ----
            last = nxt  # reuse the other scan buffer
            nc.vector.tensor_tensor(out=last[:], in0=cur[:],
                                    in1=pref[:, 0:1].to_broadcast([QP, 128]),
                                    op=mybir.AluOpType.max)
            ge0 = cpool.tile([QP, 128], F32)
            nc.vector.tensor_scalar(out=ge0[:], in0=last[:], scalar1=0.0,
                                    scalar2=None, op0=mybir.AluOpType.is_ge)
            rel4 = cpool.tile([QP, 128], F32)
            # rel + 4 = (s + 1) + 3 - last
            nc.vector.tensor_tensor(out=rel4[:], in0=svalp1, in1=last[:],
                                    op=mybir.AluOpType.subtract)
            nc.vector.tensor_scalar_add(out=rel4[:], in0=rel4[:], scalar1=3.0)
            le = cpool.tile([QP, 128], F32)
            nc.vector.tensor_scalar(out=le[:], in0=rel4[:], scalar1=1006.0,
                                    scalar2=None, op0=mybir.AluOpType.is_le)
            valid = cpool.tile([QP, 128], F32)
            nc.vector.tensor_tensor(out=valid[:], in0=ge0[:], in1=le[:],
                                    op=mybir.AluOpType.mult)
            idxf = cpool.tile([QP, 128], F32)
            # idx = base + valid * (rel4 - base)
            nc.vector.tensor_tensor(out=idxf[:], in0=rel4[:], in1=basei,
                                    op=mybir.AluOpType.subtract)
            nc.vector.tensor_tensor(out=idxf[:], in0=idxf[:], in1=valid[:],
                                    op=mybir.AluOpType.mult)
            nc.vector.tensor_tensor(out=idxf[:], in0=idxf[:], in1=basei,
                                    op=mybir.AluOpType.add)

            # ---- transpose to gather layout: idxT[p, q] = idx(token q*128+p) ----
            idxT_ps = ppool.tile([128, QP], F32, space="PSUM")
            nc.tensor.matmul(out=idxT_ps[:], lhsT=idxf[:], rhs=id32t,
                             start=True, stop=True)
            idxT = cpool.tile([128, QP], I32)
            nc.vector.tensor_copy(out=idxT[:], in_=idxT_ps[:])

            # ---- main data path: load x tile, gather emb rows, add, store ----
            for q in range(NT):
                xt = wpool.tile([128, D], F32)
                emb = wpool.tile([128, D], F32)
                nc.sync.dma_start(out=xt[:], in_=x_h[q * 128:(q + 1) * 128, :])
                nc.gpsimd.indirect_dma_start(
                    out=emb[:],
                    out_offset=None,
                    in_=table_h[:],
                    in_offset=bass.IndirectOffsetOnAxis(
                        ap=idxT[:, q:q + 1], axis=0),
                )
                nc.vector.tensor_add(out=xt[:], in0=xt[:], in1=emb[:])
                nc.sync.dma_start(out=out_h[q * 128:(q + 1) * 128, :], in_=xt[:])
    nc.compile()
    return nc


def _consts():
    s = np.arange(S, dtype=np.float32).reshape(NQ, 128)
    svalp1 = np.tile(s + 1.0, (2, 1)).astype(np.float32)           # [32,128]
    base = np.where(s < N_CTRL, s, float(ZERO_ROW))
    base = np.tile(base, (2, 1)).astype(np.float32)                # [32,128]
    return svalp1, base


def _run(inputs, trace=False, tmpdir=None):
    if trace:
        _ensure_ntff_hook()
    x = np.asarray(inputs["x"], dtype=np.float32)
    ids = np.asarray(inputs["input_ids"])
    stid = int(np.asarray(inputs["start_token_id"]))
    ctrl = np.asarray(inputs["control_emb"], dtype=np.float32)
    seq = np.asarray(inputs["sequence_emb"], dtype=np.float32)

    if "nc" not in _CACHE:
        _CACHE["nc"] = _build_bass()
    nc = _CACHE["nc"]

    table = np.concatenate(
        [ctrl, seq, np.zeros((1, D), dtype=np.float32)], axis=0)
    svalp1, base = _consts()

    pos_ok = np.arange(S) >= N_CTRL
    mask = ((ids == stid) & pos_ok[None, :]).astype(np.float32)    # [B, S]

    in_maps = []
    for i in range(N_CORES):
        b0 = i * B_SH
        xsh = np.ascontiguousarray(
            x[b0:b0 + B_SH].reshape(TOK, D))
        msh = mask[b0:b0 + B_SH].reshape(2 * NQ, 128)
        id32 = np.eye(2 * NQ, 2 * NQ, dtype=np.float32)
        ones = np.ones((2 * NQ, 1), dtype=np.float32)
        cst = np.ascontiguousarray(
            np.concatenate([msh, svalp1, base, id32, ones], axis=1))  # [32, 417]
        in_maps.append({"x": xsh, "consts": cst, "table": table})

    res = run_bass_kernel_spmd(nc, in_maps, core_ids=list(range(N_CORES)),
                               trace=trace, tmpdir=tmpdir)
    out = np.concatenate(
        [res.results[i]["out"].reshape(B_SH, S, D) for i in range(N_CORES)],
        axis=0)
    return out, res


def kernel(**inputs) -> np.ndarray:
    out, _ = _run(inputs, trace=bool(os.environ.get("BASS_TRACE")))
    return out
